# revision 3
# baseline (speedup 1.0000x reference)
"""Trainium2 Bass kernel for CropProposals (adaptive max-pool 2x2x2 over
data-dependent crops of a [4,128,24,24,24] feature map).

Sharding: core k = 2*b + h handles batch b with a load-balanced half of the
64 proposals (full 128-channel dim on SBUF partitions).  All crop bounds are
computed on the host from `corners` (tiny int math) and baked into the Bass
program as static access patterns; per-core differences live in 8
partition-id branches of one SPMD program.  Each octant pair (oz in {0,1})
of a proposal is one VectorE tensor_reduce over a strided 5-D access
pattern [C][oz][d][h][w] reducing d/h/w.
"""

import numpy as np

_B, _C, _D, _H, _W = 4, 128, 24, 24, 24
_P = 64
_NCORES = 8
_PPC = _P // 2          # proposals per core
_VOL = _D * _H * _W     # 13824
_SD, _SH, _SW = _H * _W, _W, 1   # element strides of [D,H,W] volume

_cache = {}


def _box_params(corners, scale):
    """Host-side replica of the reference bound math.

    Returns s, l, dlt arrays of shape [B, P, 3] (axis order D,H,W):
      region(o) along axis a = [ s + o*dlt , s + o*dlt + l )
    """
    c = np.asarray(corners).astype(np.int64)
    p1 = np.clip(c[:, :, 0, :] // scale, 0, 21)
    p2r = c[:, :, 1, :] // scale
    p2 = np.where(p2r - p1 >= 2, p2r, p1 + 2)
    sizes = np.array([_D, _H, _W], dtype=np.int64)
    e = np.minimum(p2, sizes)
    n = e - p1                 # crop length per axis, >= 2
    l = (n + 1) // 2           # region length (same for both regions)
    dlt = n // 2               # region-1 start offset from region-0 start
    return p1, l, dlt


def _assign_proposals(s, l, dlt):
    """Balance proposals between the two cores of each batch by estimated
    VectorE cycles (8*vol + fixed per-proposal instruction overhead)."""
    assign = []   # per batch: (idx_core0, idx_core1)
    for b in range(_B):
        vol = l[b].prod(axis=-1)
        cost = 8 * vol + 290
        order = np.argsort(-cost)
        loads = [0, 0]
        sets = [[], []]
        for p in order:
            k = 0 if (loads[0] <= loads[1] and len(sets[0]) < _PPC) or len(sets[1]) >= _PPC else 1
            sets[k].append(int(p))
            loads[k] += int(cost[p])
        assign.append((sets[0], sets[1]))
    return assign


def _build_program(s, l, dlt, assign):
    import concourse.bacc as bacc
    import concourse.mybir as mybir
    from concourse.tile import TileContext
    from concourse.ap import AP

    nc = bacc.Bacc("TRN2", target_bir_lowering=False, debug=False,
                   num_devices=_NCORES)
    x_in = nc.dram_tensor("fm", [_C, _VOL], mybir.dt.float32,
                          kind="ExternalInput")
    y_out = nc.dram_tensor("out", [_C, _PPC * 8], mybir.dt.float32,
                           kind="ExternalOutput")

    n_chunks = 6
    dpc = _D // n_chunks      # D planes per chunk

    with TileContext(nc) as tc:
        with tc.tile_pool(name="pool", bufs=1) as pool:
            xt = pool.tile([_C, _VOL], mybir.dt.float32)
            yt = pool.tile([_C, _PPC * 8], mybir.dt.float32)
            for ci in range(n_chunks):
                sl = slice(ci * dpc * _SD, (ci + 1) * dpc * _SD)
                nc.sync.dma_start(out=xt[:, sl], in_=x_in[:, sl])
            pid = nc.partition_id()
            base = xt[:]
            part_dim = list(base.ap[0])
            for k in range(_NCORES):
                b, h = k // 2, k % 2
                plist = assign[b][h]
                # issue proposals in order of max D index touched so early
                # DMA chunks unblock early reduces
                plist = sorted(plist, key=lambda p: s[b, p, 0] + dlt[b, p, 0] + l[b, p, 0])
                with tc.If(pid == k):
                    for j, p in enumerate(plist):
                        sx, sy, sz = (int(v) for v in s[b, p])
                        lx, ly, lz = (int(v) for v in l[b, p])
                        dx, dy, dz = (int(v) for v in dlt[b, p])
                        for ox in range(2):
                            for oy in range(2):
                                off = ((sx + ox * dx) * _SD
                                       + (sy + oy * dy) * _SH + sz)
                                ap = AP(base.tensor, base.offset + off,
                                        [part_dim, [dz, 2], [_SD, lx],
                                         [_SH, ly], [1, lz]])
                                col = j * 8 + ox * 4 + oy * 2
                                nc.vector.tensor_reduce(
                                    out=yt[:, col:col + 2], in_=ap,
                                    axis=mybir.AxisListType.XYZ,
                                    op=mybir.AluOpType.max)
            nc.sync.dma_start(out=y_out[:], in_=yt[:])
    nc.compile()
    return nc


def _get_program(corners, scale):
    key = (np.asarray(corners).tobytes(), int(scale))
    if key not in _cache:
        s, l, dlt = _box_params(corners, scale)
        assign = _assign_proposals(s, l, dlt)
        nc = _build_program(s, l, dlt, assign)
        _cache[key] = (nc, assign)
    return _cache[key]


def _install_ntff_shim():
    """The agent image's antenv lacks axon_hooks; recreate it so
    run_bass_kernel_spmd(trace=True) can capture NTFF profiles."""
    import sys
    import types
    try:
        import antenv.axon_hooks  # noqa: F401
        return
    except ImportError:
        pass
    try:
        from trn_agent_boot.trn_boot import _ntff_profile_via_ctypes
        hook = _ntff_profile_via_ctypes("/opt/axon/libaxon_pjrt.so")
        mod = types.ModuleType("antenv.axon_hooks")
        mod._hook = hook
        mod.get_axon_ntff_profile_hook = lambda: mod._hook

        def _set(h):
            mod._hook = h

        mod.set_axon_ntff_profile_hook = _set
        sys.modules["antenv.axon_hooks"] = mod
        import antenv
        antenv.axon_hooks = mod
    except Exception:
        pass


def _run(fm, corners, scale, trace=False, trace_cores=None):
    from concourse.bass_utils import run_bass_kernel_spmd
    if trace:
        _install_ntff_shim()

    fm = np.ascontiguousarray(np.asarray(fm, dtype=np.float32))
    scale = int(scale)
    nc, assign = _get_program(corners, scale)

    in_maps = []
    for k in range(_NCORES):
        b = k // 2
        in_maps.append({"fm": fm[b].reshape(_C, _VOL)})

    kwargs = {}
    if trace:
        kwargs.update(trace=True,
                      trace_cores=trace_cores or list(range(_NCORES)))
    res = run_bass_kernel_spmd(nc, in_maps, list(range(_NCORES)), **kwargs)

    out = np.empty((_B, _P, _C, 2, 2, 2), dtype=np.float32)
    # recompute the exact issue order used at build time
    s, l, dlt = _box_params(corners, scale)
    for k in range(_NCORES):
        b, h = k // 2, k % 2
        plist = assign[b][h]
        plist = sorted(plist, key=lambda p: s[b, p, 0] + dlt[b, p, 0] + l[b, p, 0])
        y = res.results[k]["out"]                       # [C, PPC*8]
        y = y.reshape(_C, _PPC, 2, 2, 2)
        for j, p in enumerate(plist):
            out[b, p] = y[:, j]
    return out, getattr(res, "exec_time_ns", None)


def kernel(fm, corners, scale=4):
    out, _ = _run(fm, corners, scale, trace=False)
    return out


# revision 4
# speedup vs baseline: 1.0148x; 1.0148x over previous
"""Trainium2 Bass kernel for CropProposals (adaptive max-pool 2x2x2 over
data-dependent crops of a [4,128,24,24,24] feature map).

Sharding: core k = 2*b + h handles batch b with a load-balanced half of the
64 proposals (full 128-channel dim on SBUF partitions).  All crop bounds are
computed on the host from `corners` (tiny int math) and baked into the Bass
program as static access patterns; per-core differences live in 8
partition-id branches of one SPMD program.  Each octant pair (oz in {0,1})
of a proposal is one VectorE tensor_reduce over a strided 5-D access
pattern [C][oz][d][h][w] reducing d/h/w.
"""

import numpy as np

_B, _C, _D, _H, _W = 4, 128, 24, 24, 24
_P = 64
_NCORES = 8
_PPC = _P // 2          # proposals per core
_VOL = _D * _H * _W     # 13824
_SD, _SH, _SW = _H * _W, _W, 1   # element strides of [D,H,W] volume

_cache = {}


def _box_params(corners, scale):
    """Host-side replica of the reference bound math.

    Returns s, l, dlt arrays of shape [B, P, 3] (axis order D,H,W):
      region(o) along axis a = [ s + o*dlt , s + o*dlt + l )
    """
    c = np.asarray(corners).astype(np.int64)
    p1 = np.clip(c[:, :, 0, :] // scale, 0, 21)
    p2r = c[:, :, 1, :] // scale
    p2 = np.where(p2r - p1 >= 2, p2r, p1 + 2)
    sizes = np.array([_D, _H, _W], dtype=np.int64)
    e = np.minimum(p2, sizes)
    n = e - p1                 # crop length per axis, >= 2
    l = (n + 1) // 2           # region length (same for both regions)
    dlt = n // 2               # region-1 start offset from region-0 start
    return p1, l, dlt


def _assign_proposals(s, l, dlt):
    """Balance proposals between the two cores of each batch by estimated
    VectorE cycles (8*vol + fixed per-proposal instruction overhead)."""
    assign = []   # per batch: (idx_core0, idx_core1)
    for b in range(_B):
        vol = l[b].prod(axis=-1)
        cost = 8 * vol + 290
        order = np.argsort(-cost)
        loads = [0, 0]
        sets = [[], []]
        for p in order:
            k = 0 if (loads[0] <= loads[1] and len(sets[0]) < _PPC) or len(sets[1]) >= _PPC else 1
            sets[k].append(int(p))
            loads[k] += int(cost[p])
        assign.append((sets[0], sets[1]))
    return assign


def _build_program(s, l, dlt, assign):
    import concourse.bacc as bacc
    import concourse.mybir as mybir
    from concourse.tile import TileContext
    from concourse.ap import AP

    nc = bacc.Bacc("TRN2", target_bir_lowering=False, debug=False,
                   num_devices=_NCORES)
    x_in = nc.dram_tensor("fm", [_C, _VOL], mybir.dt.float32,
                          kind="ExternalInput")
    y_out = nc.dram_tensor("out", [_C, _PPC * 8], mybir.dt.float32,
                           kind="ExternalOutput")

    n_chunks = 6
    dpc = _D // n_chunks      # D planes per chunk

    with TileContext(nc) as tc:
        with tc.tile_pool(name="pool", bufs=1) as pool:
            xt = pool.tile([_C, _VOL], mybir.dt.float32)
            yt = pool.tile([_C, _PPC * 8], mybir.dt.float32)
            for ci in range(n_chunks):
                sl = slice(ci * dpc * _SD, (ci + 1) * dpc * _SD)
                nc.sync.dma_start(out=xt[:, sl], in_=x_in[:, sl])
            # restrict the partition-id register (and therefore the If
            # branches) to the Vector engine: the other 4 engines then skip
            # the whole branch cascade instead of walking 8 blocks of
            # event-semaphore choreography (~13us on the measured trace)
            pid = nc.partition_id(engines=(mybir.EngineType.DVE,))
            base = xt[:]
            part_dim = list(base.ap[0])
            for k in range(_NCORES):
                b, h = k // 2, k % 2
                plist = assign[b][h]
                # issue proposals in order of max D index touched so early
                # DMA chunks unblock early reduces
                plist = sorted(plist, key=lambda p: s[b, p, 0] + dlt[b, p, 0] + l[b, p, 0])
                with tc.If(pid == k):
                    for j, p in enumerate(plist):
                        sx, sy, sz = (int(v) for v in s[b, p])
                        lx, ly, lz = (int(v) for v in l[b, p])
                        dx, dy, dz = (int(v) for v in dlt[b, p])
                        for ox in range(2):
                            for oy in range(2):
                                off = ((sx + ox * dx) * _SD
                                       + (sy + oy * dy) * _SH + sz)
                                ap = AP(base.tensor, base.offset + off,
                                        [part_dim, [dz, 2], [_SD, lx],
                                         [_SH, ly], [1, lz]])
                                col = j * 8 + ox * 4 + oy * 2
                                nc.vector.tensor_reduce(
                                    out=yt[:, col:col + 2], in_=ap,
                                    axis=mybir.AxisListType.XYZ,
                                    op=mybir.AluOpType.max)
            nc.sync.dma_start(out=y_out[:], in_=yt[:])
    nc.compile()
    return nc


def _get_program(corners, scale):
    key = (np.asarray(corners).tobytes(), int(scale))
    if key not in _cache:
        s, l, dlt = _box_params(corners, scale)
        assign = _assign_proposals(s, l, dlt)
        nc = _build_program(s, l, dlt, assign)
        _cache[key] = (nc, assign)
    return _cache[key]


def _install_ntff_shim():
    """The agent image's antenv lacks axon_hooks; recreate it so
    run_bass_kernel_spmd(trace=True) can capture NTFF profiles."""
    import sys
    import types
    try:
        import antenv.axon_hooks  # noqa: F401
        return
    except ImportError:
        pass
    try:
        from trn_agent_boot.trn_boot import _ntff_profile_via_ctypes
        hook = _ntff_profile_via_ctypes("/opt/axon/libaxon_pjrt.so")
        mod = types.ModuleType("antenv.axon_hooks")
        mod._hook = hook
        mod.get_axon_ntff_profile_hook = lambda: mod._hook

        def _set(h):
            mod._hook = h

        mod.set_axon_ntff_profile_hook = _set
        sys.modules["antenv.axon_hooks"] = mod
        import antenv
        antenv.axon_hooks = mod
    except Exception:
        pass


def _run(fm, corners, scale, trace=False, trace_cores=None):
    from concourse.bass_utils import run_bass_kernel_spmd
    if trace:
        _install_ntff_shim()

    fm = np.ascontiguousarray(np.asarray(fm, dtype=np.float32))
    scale = int(scale)
    nc, assign = _get_program(corners, scale)

    in_maps = []
    for k in range(_NCORES):
        b = k // 2
        in_maps.append({"fm": fm[b].reshape(_C, _VOL)})

    kwargs = {}
    if trace:
        kwargs.update(trace=True,
                      trace_cores=trace_cores or list(range(_NCORES)))
    res = run_bass_kernel_spmd(nc, in_maps, list(range(_NCORES)), **kwargs)

    out = np.empty((_B, _P, _C, 2, 2, 2), dtype=np.float32)
    # recompute the exact issue order used at build time
    s, l, dlt = _box_params(corners, scale)
    for k in range(_NCORES):
        b, h = k // 2, k % 2
        plist = assign[b][h]
        plist = sorted(plist, key=lambda p: s[b, p, 0] + dlt[b, p, 0] + l[b, p, 0])
        y = res.results[k]["out"]                       # [C, PPC*8]
        y = y.reshape(_C, _PPC, 2, 2, 2)
        for j, p in enumerate(plist):
            out[b, p] = y[:, j]
    return out, getattr(res, "exec_time_ns", None)


def kernel(fm, corners, scale=4):
    out, _ = _run(fm, corners, scale, trace=False)
    return out
